# revision 4
# baseline (speedup 1.0000x reference)
"""MixLoRA sparse-MoE Trainium2 kernel.

Strategy: tensor-parallel over d_ff (F=4096 -> 512 per core) on 8 NeuronCores.
Every core processes all 1024 tokens for its F-slice; the down-projection
produces per-core partial sums over its F-slice which are reduced on the host.

Device layout is feature-major ("transposed"): activations are [feat, token]
so every matmul contraction axis lands on SBUF partitions with zero on-device
transposes.  Top-2 routing is computed on device from logits (softmax ratio ==
sigmoid of logit difference, exactly matching the reference's renormalized
top-2 softmax weights); per-expert LoRA deltas use a block-mask formulation:
    delta_branch = (sT * mask_branch) @ B_flat
which turns the per-token expert gather into dense rank-128 matmuls.

Heavy matmuls run in float32r (full PE rate, ~1.6e-4 rel err); the router
matmul runs in true fp32 so top-2 decisions match the fp32 reference.
"""
import sys

sys.path.insert(0, "/opt/trn_rl_repo")

from contextlib import ExitStack

import numpy as np

import concourse.tile as tile
from concourse import bacc, bass_isa, mybir
from concourse.bass_utils import run_bass_kernel_spmd

f32 = mybir.dt.float32
f32r = mybir.dt.float32r
AF = mybir.ActivationFunctionType
ALU = mybir.AluOpType
RED = bass_isa.ReduceOp

NCORES = 8
N = 1024          # tokens (B*S)
D = 1024          # hidden
F = 4096          # d_ff
FC = F // NCORES  # 512 per-core f-slice
E = 8             # experts
R = 16            # lora rank
ER = E * R        # 128
NT = 512          # token tile (free dim of matmuls)
P = 128
DT = D // P       # 8
FT = FC // P      # 4
TT = N // NT      # 2

_CACHE = {}


def _build():
    nc = bacc.Bacc("TRN2", target_bir_lowering=False, debug=False)

    xT_d = nc.dram_tensor("xT", [D, N], f32, kind="ExternalInput")
    gwT_d = nc.dram_tensor("gwT", [D, E], f32, kind="ExternalInput")
    a1t_d = nc.dram_tensor("a1t", [D, ER], f32, kind="ExternalInput")
    a3t_d = nc.dram_tensor("a3t", [D, ER], f32, kind="ExternalInput")
    w1t_d = nc.dram_tensor("w1t", [D, FC], f32, kind="ExternalInput")
    w3t_d = nc.dram_tensor("w3t", [D, FC], f32, kind="ExternalInput")
    wdt_d = nc.dram_tensor("wdt", [FC, D], f32, kind="ExternalInput")
    b1t_d = nc.dram_tensor("b1t", [ER, FC], f32, kind="ExternalInput")
    b3t_d = nc.dram_tensor("b3t", [ER, FC], f32, kind="ExternalInput")
    a2t_d = nc.dram_tensor("a2t", [FC, ER], f32, kind="ExternalInput")
    b2f_d = nc.dram_tensor("b2f", [ER, D], f32, kind="ExternalInput")
    outT_d = nc.dram_tensor("outT", [D, N], f32, kind="ExternalOutput")

    r16_np = np.zeros((E, ER), dtype=np.float32)
    for e in range(E):
        r16_np[e, e * R:(e + 1) * R] = 1.0
    r16_d = nc.inline_tensor(r16_np, name="r16")

    with tile.TileContext(nc) as tc, ExitStack() as ctx:
        sb = ctx.enter_context(tc.tile_pool(name="sb", bufs=1))
        ps = ctx.enter_context(tc.tile_pool(name="ps", bufs=2, space="PSUM"))
        # mpool opened before 'early' so it can outlive it (LIFO stack)
        mpool = ctx.enter_context(tc.tile_pool(name="mpool", bufs=1))

        def load_tall(pool, tag, shape, dram, dtype):
            t = pool.tile(shape, dtype, tag=tag)
            src = dram[:, :].rearrange("(a p) w -> p a w", p=P)
            if dtype == f32r:
                src = src.bitcast(f32r)
            nc.sync.dma_start(out=t[:], in_=src)
            return t

        # ---- persistent tiles ----
        xT = load_tall(sb, "xT", [P, DT, N], xT_d, f32r)
        r16 = sb.tile([E, ER], f32r)
        nc.sync.dma_start(out=r16[:], in_=r16_d[:, :].bitcast(f32r))
        w1t = load_tall(sb, "w1t", [P, DT, FC], w1t_d, f32r)
        w3t = load_tall(sb, "w3t", [P, DT, FC], w3t_d, f32r)
        b1t = sb.tile([ER, FC], f32r)
        nc.sync.dma_start(out=b1t[:], in_=b1t_d[:, :].bitcast(f32r))
        b3t = sb.tile([ER, FC], f32r)
        nc.sync.dma_start(out=b3t[:], in_=b3t_d[:, :].bitcast(f32r))
        a2t = load_tall(sb, "a2t", [P, FT, ER], a2t_d, f32r)
        wdt = load_tall(sb, "wdt", [P, FT, D], wdt_d, f32r)
        b2f = sb.tile([ER, D], f32r)
        mka = sb.tile([ER, N], f32)
        mkb = sb.tile([ER, N], f32)
        wa_bc = sb.tile([P, N], f32)
        wb_bc = sb.tile([P, N], f32)
        actCT = sb.tile([P, FT, N], f32r)
        zc = sb.tile([ER, N], f32r)

        def xtile(dt_, tsl):
            return xT[:, dt_, tsl]

        with tc.tile_pool(name="early", bufs=1) as early:
            gwT = load_tall(early, "gwT", [P, DT, E], gwT_d, f32)
            a1t = load_tall(early, "a1t", [P, DT, ER], a1t_d, f32r)
            a3t = load_tall(early, "a3t", [P, DT, ER], a3t_d, f32r)

            with tc.tile_pool(name="rscratch", bufs=1) as rs:
                # ======== router (fp32) ========
                logitsT = rs.tile([E, N], f32)
                for tt in range(TT):
                    tsl = slice(tt * NT, (tt + 1) * NT)
                    plg = ps.tile([E, NT], f32, tag="X")
                    for dt_ in range(DT):
                        nc.tensor.matmul(
                            out=plg[:], lhsT=gwT[:, dt_, :],
                            rhs=xtile(dt_, tsl).bitcast(f32),
                            start=(dt_ == 0), stop=(dt_ == DT - 1))
                    nc.any.tensor_copy(out=logitsT[:, tsl], in_=plg[:])

                m1 = rs.tile([E, N], f32)
                nc.gpsimd.partition_all_reduce(m1[:], logitsT[:], channels=E,
                                               reduce_op=RED.max)
                eq1 = rs.tile([E, N], f32r)
                nc.vector.tensor_tensor(out=eq1[:], in0=logitsT[:], in1=m1[:],
                                        op=ALU.is_equal)
                l2 = rs.tile([E, N], f32)
                nc.vector.scalar_tensor_tensor(
                    out=l2[:], in0=eq1[:].bitcast(f32), scalar=-1e30,
                    in1=logitsT[:], op0=ALU.mult, op1=ALU.add)
                m2 = rs.tile([E, N], f32)
                nc.gpsimd.partition_all_reduce(m2[:], l2[:], channels=E,
                                               reduce_op=RED.max)
                eq2 = rs.tile([E, N], f32r)
                nc.vector.tensor_tensor(out=eq2[:], in0=l2[:], in1=m2[:],
                                        op=ALU.is_equal)
                # wa = 1/(1+exp(m2-m1)) ; wb = 1-wa
                wa = rs.tile([1, N], f32)
                nc.vector.tensor_tensor(out=wa[:], in0=m2[0:1, :],
                                        in1=m1[0:1, :], op=ALU.subtract)
                nc.scalar.activation(out=wa[:], in_=wa[:], func=AF.Exp)
                nc.vector.tensor_scalar_add(out=wa[:], in0=wa[:], scalar1=1.0)
                nc.vector.reciprocal(out=wa[:], in_=wa[:])
                wb = rs.tile([1, N], f32)
                nc.vector.scalar_tensor_tensor(
                    out=wb[:], in0=wa[:], scalar=-1.0, in1=wa[:],
                    op0=ALU.mult, op1=ALU.bypass)
                nc.vector.tensor_scalar_add(out=wb[:], in0=wb[:], scalar1=1.0)

                nc.gpsimd.partition_broadcast(wa_bc[:], wa[:])
                nc.gpsimd.partition_broadcast(wb_bc[:], wb[:])

                # ---- replicate one-hot masks to [er, n] ----
                for tt in range(TT):
                    tsl = slice(tt * NT, (tt + 1) * NT)
                    pm = ps.tile([ER, NT], f32, tag="Y")
                    nc.tensor.matmul(out=pm[:], lhsT=r16[:], rhs=eq1[:, tsl],
                                     start=True, stop=True)
                    nc.any.tensor_copy(out=mka[:, tsl], in_=pm[:])
                    pm2 = ps.tile([ER, NT], f32, tag="Y")
                    nc.tensor.matmul(out=pm2[:], lhsT=r16[:], rhs=eq2[:, tsl],
                                     start=True, stop=True)
                    nc.any.tensor_copy(out=mkb[:, tsl], in_=pm2[:])

            # ======== LoRA-A stage + masked s (reads psum directly) ========
            m1aT = mpool.tile([ER, N], f32r, tag="m1a")
            m3aT = mpool.tile([ER, N], f32r, tag="m3a")
            m1bT = mpool.tile([ER, N], f32r, tag="m1b")
            m3bT = mpool.tile([ER, N], f32r, tag="m3b")
            for tt in range(TT):
                tsl = slice(tt * NT, (tt + 1) * NT)
                ps1 = ps.tile([ER, NT], f32, tag="D1")
                for dt_ in range(DT):
                    nc.tensor.matmul(out=ps1[:], lhsT=a1t[:, dt_, :],
                                     rhs=xtile(dt_, tsl),
                                     start=(dt_ == 0), stop=(dt_ == DT - 1))
                nc.vector.tensor_tensor(out=m1aT[:, tsl], in0=ps1[:],
                                        in1=mka[:, tsl], op=ALU.mult)
                nc.vector.tensor_tensor(out=m1bT[:, tsl], in0=ps1[:],
                                        in1=mkb[:, tsl], op=ALU.mult)
                ps3 = ps.tile([ER, NT], f32, tag="D3")
                for dt_ in range(DT):
                    nc.tensor.matmul(out=ps3[:], lhsT=a3t[:, dt_, :],
                                     rhs=xtile(dt_, tsl),
                                     start=(dt_ == 0), stop=(dt_ == DT - 1))
                nc.vector.tensor_tensor(out=m3aT[:, tsl], in0=ps3[:],
                                        in1=mka[:, tsl], op=ALU.mult)
                nc.vector.tensor_tensor(out=m3bT[:, tsl], in0=ps3[:],
                                        in1=mkb[:, tsl], op=ALU.mult)

        # ======== main loop ========
        ca_tiles = {}
        cb_tiles = {}
        with tc.tile_pool(name="work", bufs=2) as work, \
                tc.tile_pool(name="cpool", bufs=5) as cpool:
            for tt in range(TT):
                tsl = slice(tt * NT, (tt + 1) * NT)
                for ft in range(FT):
                    fsl = slice(ft * P, (ft + 1) * P)
                    pX = ps.tile([P, NT], f32, tag="X")
                    for dt_ in range(DT):
                        nc.tensor.matmul(out=pX[:], lhsT=w1t[:, dt_, fsl],
                                         rhs=xtile(dt_, tsl),
                                         start=(dt_ == 0), stop=False)
                    c1sb = work.tile([P, NT], f32, tag="c1sb")
                    nc.scalar.copy(out=c1sb[:], in_=pX[:])
                    nc.tensor.matmul(out=pX[:], lhsT=b1t[:, fsl],
                                     rhs=m1aT[:, tsl], start=False, stop=True)
                    pY = ps.tile([P, NT], f32, tag="Y")
                    for dt_ in range(DT):
                        nc.tensor.matmul(out=pY[:], lhsT=w3t[:, dt_, fsl],
                                         rhs=xtile(dt_, tsl),
                                         start=(dt_ == 0), stop=False)
                    c3sb = work.tile([P, NT], f32, tag="c3sb")
                    nc.scalar.copy(out=c3sb[:], in_=pY[:])
                    nc.tensor.matmul(out=pY[:], lhsT=b3t[:, fsl],
                                     rhs=m3aT[:, tsl], start=False, stop=True)
                    pD1 = ps.tile([P, NT], f32, tag="D1")
                    nc.tensor.matmul(out=pD1[:], lhsT=b1t[:, fsl],
                                     rhs=m1bT[:, tsl], start=True, stop=True)
                    pD3 = ps.tile([P, NT], f32, tag="D3")
                    nc.tensor.matmul(out=pD3[:], lhsT=b3t[:, fsl],
                                     rhs=m3bT[:, tsl], start=True, stop=True)

                    # branch a: ua = silu(pX)*wa ; ca = ua*pY
                    ua = work.tile([P, NT], f32, tag="ua")
                    nc.scalar.activation(out=ua[:], in_=pX[:], func=AF.Silu)
                    nc.vector.tensor_tensor(out=ua[:], in0=ua[:],
                                            in1=wa_bc[:, tsl], op=ALU.mult)
                    ca = cpool.tile([P, NT], f32r, tag="ca")
                    nc.vector.tensor_tensor(out=ca[:], in0=ua[:], in1=pY[:],
                                            op=ALU.mult)
                    # branch b: tb = c1sb+pD1 -> silu -> *wb ; vb = c3sb+pD3
                    nc.vector.tensor_tensor(out=c1sb[:], in0=c1sb[:],
                                            in1=pD1[:], op=ALU.add)
                    ub = work.tile([P, NT], f32, tag="ub")
                    nc.scalar.activation(out=ub[:], in_=c1sb[:], func=AF.Silu)
                    nc.vector.tensor_tensor(out=ub[:], in0=ub[:],
                                            in1=wb_bc[:, tsl], op=ALU.mult)
                    nc.vector.tensor_tensor(out=c3sb[:], in0=c3sb[:],
                                            in1=pD3[:], op=ALU.add)
                    cb = cpool.tile([P, NT], f32r, tag="cb")
                    nc.vector.tensor_tensor(out=cb[:], in0=ub[:], in1=c3sb[:],
                                            op=ALU.mult)
                    ca_tiles[(ft, tt)] = ca
                    cb_tiles[(ft, tt)] = cb
                    nc.vector.tensor_tensor(out=actCT[:, ft, tsl], in0=ca[:],
                                            in1=cb[:], op=ALU.add)

                # ---- LoRA-down z for this token tile ----
                pza = ps.tile([ER, NT], f32, tag="D1")
                for ft in range(FT):
                    nc.tensor.matmul(out=pza[:], lhsT=a2t[:, ft, :],
                                     rhs=ca_tiles[(ft, tt)][:],
                                     start=(ft == 0), stop=(ft == FT - 1))
                za = cpool.tile([ER, NT], f32r, tag="ca")
                nc.vector.tensor_tensor(out=za[:], in0=pza[:], in1=mka[:, tsl],
                                        op=ALU.mult)
                pzb = ps.tile([ER, NT], f32, tag="D3")
                for ft in range(FT):
                    nc.tensor.matmul(out=pzb[:], lhsT=a2t[:, ft, :],
                                     rhs=cb_tiles[(ft, tt)][:],
                                     start=(ft == 0), stop=(ft == FT - 1))
                zb = cpool.tile([ER, NT], f32r, tag="cb")
                nc.vector.tensor_tensor(out=zb[:], in0=pzb[:], in1=mkb[:, tsl],
                                        op=ALU.mult)
                nc.vector.tensor_tensor(out=zc[:, tsl], in0=za[:], in1=zb[:],
                                        op=ALU.add)

        # ======== down projection (partial over f-slice) ========
        nc.sync.dma_start(out=b2f[:], in_=b2f_d[:, :].bitcast(f32r))
        with tc.tile_pool(name="opool", bufs=3) as opool:
            for tt in range(TT):
                tsl = slice(tt * NT, (tt + 1) * NT)
                for dt_ in range(DT):
                    po = ps.tile([P, NT], f32,
                                 tag=("X" if dt_ % 2 == 0 else "Y"))
                    for ft in range(FT):
                        nc.tensor.matmul(
                            out=po[:],
                            lhsT=wdt[:, ft, dt_ * P:(dt_ + 1) * P],
                            rhs=actCT[:, ft, tsl],
                            start=(ft == 0), stop=False)
                    nc.tensor.matmul(out=po[:],
                                     lhsT=b2f[:, dt_ * P:(dt_ + 1) * P],
                                     rhs=zc[:, tsl], start=False, stop=True)
                    ot = opool.tile([P, NT], f32, tag="ot")
                    nc.any.tensor_copy(out=ot[:], in_=po[:])
                    nc.sync.dma_start(out=outT_d[dt_ * P:(dt_ + 1) * P, tsl],
                                      in_=ot[:])

    nc.compile()
    return nc


def _prep_in_maps(inputs):
    hs = np.asarray(inputs["hidden_states"], dtype=np.float32)
    gate_w = np.asarray(inputs["gate_w"], dtype=np.float32)
    w_gate = np.asarray(inputs["w_gate"], dtype=np.float32)
    w_up = np.asarray(inputs["w_up"], dtype=np.float32)
    w_down = np.asarray(inputs["w_down"], dtype=np.float32)
    A1 = np.asarray(inputs["A1"], dtype=np.float32)
    B1 = np.asarray(inputs["B1"], dtype=np.float32)
    A3 = np.asarray(inputs["A3"], dtype=np.float32)
    B3 = np.asarray(inputs["B3"], dtype=np.float32)
    A2 = np.asarray(inputs["A2"], dtype=np.float32)
    B2 = np.asarray(inputs["B2"], dtype=np.float32)

    x = hs.reshape(-1, D)
    C = np.ascontiguousarray
    xT = C(x.T)
    gwT = C(gate_w.T)
    a1t = C(A1.reshape(ER, D).T)
    a3t = C(A3.reshape(ER, D).T)
    b2f = C((2.0 * B2).transpose(0, 2, 1).reshape(ER, D))

    in_maps = []
    for c in range(NCORES):
        fsl = slice(c * FC, (c + 1) * FC)
        in_maps.append({
            "xT": xT,
            "gwT": gwT,
            "a1t": a1t,
            "a3t": a3t,
            "w1t": C(w_gate[fsl].T),
            "w3t": C(w_up[fsl].T),
            "wdt": C(w_down[:, fsl].T),
            "b1t": C((2.0 * B1[:, fsl, :]).transpose(0, 2, 1).reshape(ER, FC)),
            "b3t": C((2.0 * B3[:, fsl, :]).transpose(0, 2, 1).reshape(ER, FC)),
            "a2t": C(A2[:, :, fsl].reshape(ER, FC).T),
            "b2f": b2f,
        })
    return in_maps, hs.shape


def kernel(**inputs):
    if "nc" not in _CACHE:
        _CACHE["nc"] = _build()
    nc = _CACHE["nc"]
    in_maps, (B, S, _) = _prep_in_maps(inputs)
    res = run_bass_kernel_spmd(nc, in_maps, list(range(NCORES)))
    acc = np.zeros((D, N), dtype=np.float64)
    for c in range(NCORES):
        acc += res.results[c]["outT"]
    return np.ascontiguousarray(acc.T).astype(np.float32).reshape(B, S, D)
